# revision 7
# baseline (speedup 1.0000x reference)
"""Trainium2 Bass kernel for nn_CrossAttention_51539607552970.

Sharding: 8 cores = 2 (batch) x 4 (GQA kv-head groups). Each core computes
4 query heads + its single kv head for one batch element, producing a
partial output (its head-group's contribution through wo); the host sums
the 4 partials per batch element (tensor-parallel unshard).

Host passes x/c transposed as per-block SBUF images ([128, 16*512] bf16,
one contiguous 2MB DMA per 512-token block) and weights as SBUF images.
All matmuls run in bf16 (full PE rate, f32 PSUM accumulate).

Schedule: Q-projection (A) and KV-projection (B) blocks interleave
(A0 B0 A1 B1 A2 B2 B3 A3) with DMAs issued in need-time order; compute
starts ~1us after the preamble.  Attention (C) runs single-head
pipelines; the output projection (wo) of the previous query block is
interleaved one matmul per kt step as PE filler so exp (scalar) latency
never stalls the PE; the final block's wo rotates across three PSUM
tags to avoid copy-wait stalls.  PSUM: catt 2 + csum 1 + st 3 + wop 1 +
tp 1 = 8 banks.
"""

import sys

sys.path.insert(0, "/opt/trn_rl_repo")

import ml_dtypes
import numpy as np

import concourse.bass as bass
import concourse.mybir as mybir
import concourse.tile as tile
from concourse import bacc
from concourse.bass_utils import run_bass_kernel_spmd
from concourse.masks import make_identity

F32 = mybir.dt.float32
BF16 = mybir.dt.bfloat16
AF = mybir.ActivationFunctionType
OP = mybir.AluOpType

# Problem constants (hardcoded per contract).
B, S, L = 2, 2048, 2048
H, KVH, D = 16, 4, 128
HID = H * D
EPS = 1e-6
SCALE = 1.0 / np.sqrt(D)

NH = 4           # query heads per core
P = 128          # partitions
HC = HID // P    # 16 hid chunks
KC = L // P      # 16 key chunks
PB = 512         # projection block width (tokens)
AB = 512         # attention block width (queries)
NPB = S // PB    # 4
NAB = S // AB    # 4

_compiled = None


def _build():
    nc = bacc.Bacc("TRN2", num_devices=8)

    # Per-block SBUF images: [128, HC*512] bf16, contiguous.
    xT = nc.dram_tensor("xT", [NPB, P, HC * PB], BF16, kind="ExternalInput")
    cT = nc.dram_tensor("cT", [4, P, HC * 512], BF16, kind="ExternalInput")
    wq = nc.dram_tensor("wq", [HC, P, NH * D], BF16, kind="ExternalInput")
    wk = nc.dram_tensor("wk", [P, HC * D], BF16, kind="ExternalInput")
    wv = nc.dram_tensor("wv", [P, HC * D], BF16, kind="ExternalInput")
    wo = nc.dram_tensor("wo", [P, NH * HID], BF16, kind="ExternalInput")
    nqw = nc.dram_tensor("nqw", [P, 1], F32, kind="ExternalInput")
    nkw = nc.dram_tensor("nkw", [P, 1], F32, kind="ExternalInput")
    out = nc.dram_tensor("out", [S, HID], F32, kind="ExternalOutput")

    with nc.allow_low_precision(reason="bf16 matmul inputs"), \
         tile.TileContext(nc) as tc:
        with tc.tile_pool(name="consts", bufs=1) as consts, \
             tc.tile_pool(name="weights", bufs=1) as weights, \
             tc.tile_pool(name="stream", bufs=2) as stream, \
             tc.tile_pool(name="stream0", bufs=1) as stream0, \
             tc.tile_pool(name="kv", bufs=1) as kvpool, \
             tc.tile_pool(name="xqt", bufs=1) as xqtpool, \
             tc.tile_pool(name="small", bufs=2) as small, \
             tc.tile_pool(name="esbp", bufs=4) as esbp, \
             tc.tile_pool(name="outp", bufs=3) as outp, \
             tc.tile_pool(name="psum", bufs=1, space="PSUM") as psum:

            # ---- constants (no DMA deps) ----
            ones_f = consts.tile([P, P], F32)
            nc.vector.memset(ones_f[:], 1.0)
            ones_b = consts.tile([P, P], BF16)
            nc.scalar.copy(ones_b[:], ones_f[:])
            ident_f = consts.tile([P, P], F32)
            make_identity(nc, ident_f)
            ident = consts.tile([P, P], BF16)
            nc.scalar.copy(ident[:], ident_f[:])
            eps_sb = consts.tile([P, 1], F32)
            nc.vector.memset(eps_sb[:], EPS)

            nqw_sb = consts.tile([P, 1], F32)
            nkw_sb = consts.tile([P, 1], F32)
            nc.sync.dma_start(nqw_sb[:], nqw[:])
            nc.sync.dma_start(nkw_sb[:], nkw[:])

            # ---- weights ----
            wq_cs = [weights.tile([P, NH * D], BF16, name=f"wqc{hc}")
                     for hc in range(HC)]
            wk_sb = weights.tile([P, HC * D], BF16)
            wv_sb = weights.tile([P, HC * D], BF16)
            wo_sb = weights.tile([P, NH * HID], BF16)

            # ---- persistent activations ----
            kT_sb = kvpool.tile([P, L], BF16)              # [D, keys]
            v_sb = kvpool.tile([P, KC * D], BF16)          # kt-th blk [keys, D]
            xqT_list = [xqtpool.tile([P, S], BF16, name=f"xqT{h}")
                        for h in range(NH)]

            # PSUM tags (static banks): catt 2 + csum 1 + st 3 + wop 1 + tp 1
            def t_catt(nm):
                return psum.tile([P, 512], F32, name=nm, tag="catt", bufs=2)

            def t_csum(nm):
                return psum.tile([P, 512], F32, name=nm, tag="csum", bufs=1)

            def t_st(nm):
                return psum.tile([P, 512], F32, name=nm, tag="st", bufs=3)

            def t_wop(nm):
                return psum.tile([P, 512], F32, name=nm, tag="wop", bufs=1)

            def t_tp(nm):
                return psum.tile([P, 512], F32, name=nm, tag="tp", bufs=1)

            # ---------- phase-A block: Q projection for one pb ----------
            def emit_A(pb, xt_ap, extra_pe=()):
                # two 2-head passes so the norm chain of pass1 overlaps
                # pass2's matmuls (and pass2's norm overlaps the next block)
                extra_pe = list(extra_pe)
                slots = ([3, 7, 11, 15] if len(extra_pe) <= 4
                         else [1, 3, 5, 7, 9, 11, 13, 15])
                for hp in range(2):
                    hs = [2 * hp, 2 * hp + 1]
                    qpss = {h: (t_catt(f"qps{h}") if hp == 0
                                else t_csum(f"qps{h}") if h == 2
                                else t_tp(f"qps{h}")) for h in hs}
                    for hc in range(HC):
                        for h in hs:
                            nc.tensor.matmul(
                                qpss[h][:],
                                wq_cs[hc][:, h * D:(h + 1) * D],
                                xt_ap(hc),
                                start=(hc == 0), stop=(hc == HC - 1))
                        if extra_pe and hp == 0 and hc in slots:
                            extra_pe.pop(0)()
                    for h in hs:
                        qps = qpss[h]
                        qsq = small.tile([P, PB], BF16, name="qsq", tag="sq")
                        nc.scalar.square(qsq[:], qps[:])
                        qsum = t_wop("qsum")
                        nc.tensor.matmul(qsum[:], ones_b[:], qsq[:],
                                         start=True, stop=True)
                        qrs = small.tile([P, PB], F32, name="qrs", tag="rs")
                        nc.scalar.activation(qrs[:], qsum[:], AF.Sqrt,
                                             bias=eps_sb[:], scale=1.0 / D)
                        qrr = small.tile([P, PB], F32, name="qrr", tag="rr")
                        nc.vector.reciprocal_approx_fast(out=qrr[:], in_=qrs[:])
                        nc.vector.scalar_tensor_tensor(
                            out=xqT_list[h][:, pb * PB:(pb + 1) * PB],
                            in0=qps[:], scalar=nqw_sb[:], in1=qrr[:],
                            op0=OP.mult, op1=OP.mult)

            # ---------- phase-B block: K/V projection for one kcol ----------
            # returns closures emitting the 4 deferred V-transposes
            def emit_B(kcol, ct_q):
                kps = t_st("kps")
                vps = t_st("vps")
                for hc in range(HC):
                    ct_ap = ct_q[hc // 4][:, (hc % 4) * 512:(hc % 4 + 1) * 512]
                    nc.tensor.matmul(kps[:], wk_sb[:, hc * D:(hc + 1) * D],
                                     ct_ap,
                                     start=(hc == 0), stop=(hc == HC - 1))
                    nc.tensor.matmul(vps[:], wv_sb[:, hc * D:(hc + 1) * D],
                                     ct_ap,
                                     start=(hc == 0), stop=(hc == HC - 1))
                vT_sb = small.tile([P, 512], BF16, name="vT", tag="vT")
                nc.vector.tensor_copy(vT_sb[:], vps[:])
                ksq = small.tile([P, 512], BF16, name="ksq", tag="sq")
                nc.scalar.square(ksq[:], kps[:])
                ksum = t_wop("ksum")
                nc.tensor.matmul(ksum[:], ones_b[:], ksq[:],
                                 start=True, stop=True)
                krs = small.tile([P, 512], F32, name="krs", tag="rs")
                nc.scalar.activation(krs[:], ksum[:], AF.Sqrt,
                                     bias=eps_sb[:], scale=1.0 / D)
                krr = small.tile([P, 512], F32, name="krr", tag="rr")
                nc.vector.reciprocal_approx_fast(out=krr[:], in_=krs[:])
                nc.vector.scalar_tensor_tensor(
                    out=kT_sb[:, kcol * 512:(kcol + 1) * 512], in0=kps[:],
                    scalar=nkw_sb[:], in1=krr[:], op0=OP.mult, op1=OP.mult)

                def mk(j):
                    def transpose_one():
                        kt = kcol * 4 + j
                        tp = psum.tile([P, P], BF16, name="tp", tag="tp",
                                       bufs=1)
                        nc.tensor.transpose(tp[:],
                                            vT_sb[:, j * P:(j + 1) * P],
                                            ident[:])
                        nc.vector.tensor_copy(v_sb[:, kt * D:(kt + 1) * D],
                                              tp[:])
                    return transpose_one
                return [mk(j) for j in range(4)]

            # =========== interleaved A/B with need-ordered DMA ===========
            # Dual hwdge queues: ct/weights on the scalar queue, xt/out on
            # the Sync queue -> ~2x aggregate DMA issue rate pre-C.
            xt0_tiles = []
            for hc in range(HC):
                nc.scalar.dma_start(wq_cs[hc][:], wq[hc])
                t = stream0.tile([P, PB], BF16, name="xt0", tag="xstream0")
                nc.sync.dma_start(t[:], xT[0, :, hc * PB:(hc + 1) * PB])
                xt0_tiles.append(t)
            nc.scalar.dma_start(wk_sb[:], wk[:])
            nc.scalar.dma_start(wv_sb[:], wv[:])

            def ct_dma(kcol):
                qs = []
                for q in range(4):
                    t = stream.tile([P, 4 * 512], BF16, name="ctq", tag="ct",
                                    bufs=6)
                    nc.scalar.dma_start(
                        t[:], cT[kcol][:, q * 2048:(q + 1) * 2048])
                    qs.append(t)
                return qs

            def xt_dma(pb):
                qs = []
                for q in range(4):
                    t = stream.tile([P, 4 * 512], BF16, name="xtq", tag="xt",
                                    bufs=6)
                    nc.sync.dma_start(
                        t[:], xT[pb][:, q * 2048:(q + 1) * 2048])
                    qs.append(t)
                return qs

            def xt_ap_of(qs):
                return lambda hc: qs[hc // 4][:, (hc % 4) * PB:
                                              (hc % 4 + 1) * PB]

            ct0 = ct_dma(0)
            emit_A(0, lambda hc: xt0_tiles[hc][:])
            xt1 = xt_dma(1)
            ct1 = ct_dma(1)
            tr0 = emit_B(0, ct0)
            emit_A(1, xt_ap_of(xt1), extra_pe=tr0)
            xt2 = xt_dma(2)
            ct2 = ct_dma(2)
            tr1 = emit_B(1, ct1)
            emit_A(2, xt_ap_of(xt2), extra_pe=tr1)
            ct3 = ct_dma(3)
            tr2 = emit_B(2, ct2)
            xt3 = xt_dma(3)
            nc.sync.dma_start(wo_sb[:], wo[:])
            tr3 = emit_B(3, ct3)
            emit_A(3, xt_ap_of(xt3), extra_pe=tr2 + tr3)

            # =========== Phase C: attention + wo (pipelined) ===========
            prev = None  # (q0, attn_map) of previous ab awaiting wo

            def wo_filler_gen(q0p, attn_map, tags=(t_wop,)):
                """Yields once per emitted wo matmul; every 4th closes a
                (qs, ht) chunk with copy + DMA out."""
                ci = 0
                for qs in range(4):
                    for ht in range(4):
                        wop = tags[ci % len(tags)]("wop")
                        ci += 1
                        for h in range(NH):
                            nc.tensor.matmul(
                                wop[:],
                                attn_map[h][:, qs * P:(qs + 1) * P],
                                wo_sb[:, h * HID + ht * 512:
                                      h * HID + (ht + 1) * 512],
                                start=(h == 0), stop=(h == NH - 1))
                            yield
                        ot = outp.tile([P, 512], F32, name="ot", tag="ot")
                        nc.vector.tensor_copy(ot[:], wop[:])
                        nc.sync.dma_start(
                            out[q0p + qs * P: q0p + (qs + 1) * P,
                                ht * 512:(ht + 1) * 512], ot[:])

            for ab in range(NAB):
                q0 = ab * AB
                filler = (wo_filler_gen(*prev) if prev is not None else None)
                attn_map = {}
                for h in range(NH):
                    attps = t_catt(f"attps{h}")
                    sumps = t_csum(f"sumps{h}")
                    for kt in range(KC):
                        st = t_st("st")
                        nc.tensor.matmul(st[:],
                                         kT_sb[:, kt * P:(kt + 1) * P],
                                         xqT_list[h][:, q0:q0 + AB],
                                         start=True, stop=True)
                        e = esbp.tile([P, AB], BF16, name="e", tag="e")
                        nc.scalar.activation(e[:], st[:], AF.Exp)
                        nc.tensor.matmul(sumps[:], ones_b[:], e[:],
                                         start=(kt == 0), stop=(kt == KC - 1))
                        nc.tensor.matmul(attps[:],
                                         v_sb[:, kt * D:(kt + 1) * D],
                                         e[:],
                                         start=(kt == 0), stop=(kt == KC - 1))
                        if filler is not None:
                            next(filler, None)
                    rr = small.tile([P, AB], F32, name="arr", tag="arr")
                    nc.vector.reciprocal_approx_fast(out=rr[:], in_=sumps[:])
                    attn = small.tile([P, AB], BF16, name="attn",
                                      tag=f"attn{h}", bufs=2)
                    nc.vector.tensor_tensor(out=attn[:], in0=attps[:],
                                            in1=rr[:], op=OP.mult)
                    attn_map[h] = attn
                if filler is not None:
                    for _ in filler:  # drain any remainder
                        pass
                prev = (q0, attn_map)

            # final ab's wo: rotate across three tags (no next block to
            # interleave into; avoids per-chunk copy-wait stalls)
            for _ in wo_filler_gen(prev[0], prev[1],
                                   tags=(t_wop, t_st, t_csum)):
                pass

    nc.compile()
    return nc


def _get_compiled():
    global _compiled
    if _compiled is None:
        _compiled = _build()
    return _compiled


def _to_sbuf_images(aT):
    """[HID, S] f32 -> [NPB, 128, HC*512] bf16 (per-block SBUF images)."""
    t = aT.reshape(HC, P, NPB, PB).transpose(2, 1, 0, 3).reshape(
        NPB, P, HC * PB)
    return np.ascontiguousarray(t.astype(ml_dtypes.bfloat16))


def _weight_image(w, ncols):
    """[HC*P rows, ncols] -> SBUF image [128, HC*ncols] bf16."""
    nchunk = w.shape[0] // P
    img = w.reshape(nchunk, P, ncols).transpose(1, 0, 2).reshape(P, nchunk * ncols)
    return np.ascontiguousarray(img.astype(ml_dtypes.bfloat16))


def _shard_inputs(x, c, wq, wkv, wo, norm_q_w, norm_k_w):
    x = np.asarray(x, np.float32)
    c = np.asarray(c, np.float32)
    wq = np.asarray(wq, np.float32)
    wkv = np.asarray(wkv, np.float32)
    wo = np.asarray(wo, np.float32)
    nqw = (np.asarray(norm_q_w, np.float32) * np.float32(SCALE)).reshape(P, 1)
    nkw = np.asarray(norm_k_w, np.float32).reshape(P, 1).copy()

    xTs = [_to_sbuf_images(x[b].T) for b in range(B)]
    cTs = [_to_sbuf_images(c[b].T) for b in range(B)]
    in_maps = []
    for core in range(8):
        b, g = core // 4, core % 4
        blk = wkv[:, g * 256:(g + 1) * 256]
        wq_sh = wq[:, g * 512:(g + 1) * 512]
        in_maps.append({
            "xT": xTs[b],
            "cT": cTs[b],
            "wq": np.ascontiguousarray(
                wq_sh.reshape(HC, P, NH * D).astype(ml_dtypes.bfloat16)),
            "wk": _weight_image(np.ascontiguousarray(blk[:, 0::2]), D),
            "wv": _weight_image(np.ascontiguousarray(blk[:, 1::2]), D),
            "wo": _weight_image(wo[g * 512:(g + 1) * 512, :], HID),
            "nqw": nqw,
            "nkw": nkw,
        })
    return in_maps


def run_sharded(inputs, trace=False, trace_cores=None):
    """Run the SPMD kernel; returns (full_output, BassKernelResults)."""
    nc = _get_compiled()
    in_maps = _shard_inputs(**inputs)
    res = run_bass_kernel_spmd(nc, in_maps, core_ids=list(range(8)),
                               trace=trace, trace_cores=trace_cores)
    parts = [r["out"] for r in res.results]
    full = np.empty((B, S, HID), np.float32)
    for b in range(B):
        full[b] = np.sum(np.stack([parts[4 * b + g] for g in range(4)], 0),
                         axis=0, dtype=np.float64).astype(np.float32)
    return full, res


def kernel(**inputs) -> np.ndarray:
    out, _ = run_sharded(inputs, trace=False)
    return out


# revision 10
# speedup vs baseline: 1.3201x; 1.3201x over previous
"""Trainium2 Bass kernel for nn_CrossAttention_51539607552970.

Sharding: 8 cores = 2 (batch) x 4 (GQA kv-head groups). Each core computes
4 query heads + its single kv head for one batch element, producing a
partial output (its head-group's contribution through wo); the host sums
the 4 partials per batch element (tensor-parallel unshard).

Host passes x/c transposed as per-block SBUF images ([128, 16*512] bf16,
one contiguous 2MB DMA per 512-token block) and weights as SBUF images.
All matmuls run in bf16 (full PE rate, f32 PSUM accumulate).

Schedule: Q-projection (A) and KV-projection (B) blocks interleave
(A0 B0 A1 B1 A2 B2 B3 A3) with DMAs issued in need-time order; compute
starts ~1us after the preamble.  Attention (C) runs single-head
pipelines; the output projection (wo) of the previous query block is
interleaved one matmul per kt step as PE filler so exp (scalar) latency
never stalls the PE; the final block's wo rotates across three PSUM
tags to avoid copy-wait stalls.  PSUM: catt 2 + csum 1 + st 3 + wop 1 +
tp 1 = 8 banks.
"""

import sys

sys.path.insert(0, "/opt/trn_rl_repo")

import ml_dtypes
import numpy as np

import concourse.bass as bass
import concourse.mybir as mybir
import concourse.tile as tile
from concourse import bacc
from concourse.bass_utils import run_bass_kernel_spmd
from concourse.masks import make_identity

F32 = mybir.dt.float32
BF16 = mybir.dt.bfloat16
AF = mybir.ActivationFunctionType
OP = mybir.AluOpType

# Problem constants (hardcoded per contract).
B, S, L = 2, 2048, 2048
H, KVH, D = 16, 4, 128
HID = H * D
EPS = 1e-6
SCALE = 1.0 / np.sqrt(D)

NH = 4           # query heads per core
P = 128          # partitions
HC = HID // P    # 16 hid chunks
KC = L // P      # 16 key chunks
PB = 512         # projection block width (tokens)
AB = 512         # attention block width (queries)
NPB = S // PB    # 4
NAB = S // AB    # 4

_compiled = None


def _build():
    nc = bacc.Bacc("TRN2", num_devices=8)

    # Per-block SBUF images: [128, HC*512] bf16, contiguous.
    xT = nc.dram_tensor("xT", [NPB, P, HC * PB], BF16, kind="ExternalInput")
    cT = nc.dram_tensor("cT", [4, P, HC * 512], BF16, kind="ExternalInput")
    wq = nc.dram_tensor("wq", [4, P, 4 * NH * D], BF16, kind="ExternalInput")
    wk = nc.dram_tensor("wk", [P, HC * D], BF16, kind="ExternalInput")
    wv = nc.dram_tensor("wv", [P, HC * D], BF16, kind="ExternalInput")
    wo = nc.dram_tensor("wo", [P, NH * HID], BF16, kind="ExternalInput")
    nqw = nc.dram_tensor("nqw", [P, 1], F32, kind="ExternalInput")
    nkw = nc.dram_tensor("nkw", [P, 1], F32, kind="ExternalInput")
    out = nc.dram_tensor("out", [S, HID], F32, kind="ExternalOutput")

    with nc.allow_low_precision(reason="bf16 matmul inputs"), \
         tile.TileContext(nc) as tc:
        with tc.tile_pool(name="consts", bufs=1) as consts, \
             tc.tile_pool(name="weights", bufs=1) as weights, \
             tc.tile_pool(name="stream", bufs=2) as stream, \
             tc.tile_pool(name="stream0", bufs=1) as stream0, \
             tc.tile_pool(name="kv", bufs=1) as kvpool, \
             tc.tile_pool(name="xqt", bufs=1) as xqtpool, \
             tc.tile_pool(name="small", bufs=2) as small, \
             tc.tile_pool(name="esbp", bufs=4) as esbp, \
             tc.tile_pool(name="outp", bufs=3) as outp, \
             tc.tile_pool(name="psum", bufs=1, space="PSUM") as psum:

            # ---- constants (no DMA deps) ----
            ones_f = consts.tile([P, P], F32)
            nc.vector.memset(ones_f[:], 1.0)
            ones_b = consts.tile([P, P], BF16)
            nc.scalar.copy(ones_b[:], ones_f[:])
            ident_f = consts.tile([P, P], F32)
            make_identity(nc, ident_f)
            ident = consts.tile([P, P], BF16)
            nc.scalar.copy(ident[:], ident_f[:])
            eps_sb = consts.tile([P, 1], F32)
            nc.vector.memset(eps_sb[:], EPS)

            nqw_sb = consts.tile([P, 1], F32)
            nkw_sb = consts.tile([P, 1], F32)
            nc.sync.dma_start(nqw_sb[:], nqw[:])
            nc.sync.dma_start(nkw_sb[:], nkw[:])

            # ---- weights ----
            wq_qs = [weights.tile([P, 4 * NH * D], BF16, name=f"wqq{j}")
                     for j in range(4)]
            wk_sb = weights.tile([P, HC * D], BF16)
            wv_sb = weights.tile([P, HC * D], BF16)
            wo_sb = weights.tile([P, NH * HID], BF16)

            # ---- persistent activations ----
            kT_sb = kvpool.tile([P, L], BF16)              # [D, keys]
            v_sb = kvpool.tile([P, KC * D], BF16)          # kt-th blk [keys, D]
            xqT_list = [xqtpool.tile([P, S], BF16, name=f"xqT{h}")
                        for h in range(NH)]

            # PSUM tags (static banks): catt 2 + csum 1 + st 3 + wop 1 + tp 1
            def t_catt(nm):
                return psum.tile([P, 512], F32, name=nm, tag="catt", bufs=2)

            def t_csum(nm):
                return psum.tile([P, 512], F32, name=nm, tag="csum", bufs=1)

            def t_st(nm):
                return psum.tile([P, 512], F32, name=nm, tag="st", bufs=3)

            def t_wop(nm):
                return psum.tile([P, 512], F32, name=nm, tag="wop", bufs=1)

            def t_tp(nm):
                return psum.tile([P, 512], F32, name=nm, tag="tp", bufs=1)

            # ---------- phase-A block: Q projection for one pb ----------
            def emit_A(pb, xt_ap, extra_pe=()):
                # two 2-head passes so the norm chain of pass1 overlaps
                # pass2's matmuls (and pass2's norm overlaps the next block)
                extra_pe = list(extra_pe)
                slots = ([3, 7, 11, 15] if len(extra_pe) <= 4
                         else [1, 3, 5, 7, 9, 11, 13, 15])
                for hp in range(2):
                    hs = [2 * hp, 2 * hp + 1]
                    qpss = {h: (t_catt(f"qps{h}") if hp == 0
                                else t_csum(f"qps{h}") if h == 2
                                else t_tp(f"qps{h}")) for h in hs}
                    for hc in range(HC):
                        for h in hs:
                            wq_ap = wq_qs[hc // 4][
                                :, (hc % 4) * 512 + h * D:
                                   (hc % 4) * 512 + (h + 1) * D]
                            nc.tensor.matmul(
                                qpss[h][:], wq_ap, xt_ap(hc),
                                start=(hc == 0), stop=(hc == HC - 1))
                        if extra_pe and hp == 0 and hc in slots:
                            extra_pe.pop(0)()
                    for h in hs:
                        qps = qpss[h]
                        qsq = small.tile([P, PB], BF16, name="qsq", tag="sq")
                        nc.scalar.square(qsq[:], qps[:])
                        qsum = t_wop("qsum")
                        nc.tensor.matmul(qsum[:], ones_b[:], qsq[:],
                                         start=True, stop=True)
                        qrs = small.tile([P, PB], F32, name="qrs", tag="rs")
                        nc.scalar.activation(qrs[:], qsum[:], AF.Sqrt,
                                             bias=eps_sb[:], scale=1.0 / D)
                        qrr = small.tile([P, PB], F32, name="qrr", tag="rr")
                        nc.vector.reciprocal_approx_fast(out=qrr[:], in_=qrs[:])
                        nc.vector.scalar_tensor_tensor(
                            out=xqT_list[h][:, pb * PB:(pb + 1) * PB],
                            in0=qps[:], scalar=nqw_sb[:], in1=qrr[:],
                            op0=OP.mult, op1=OP.mult)

            # ---------- phase-B block: K/V projection for one kcol ----------
            # returns closures emitting the 4 deferred V-transposes
            def emit_B(kcol, ct_q):
                kps = t_st("kps")
                vps = t_st("vps")
                for hc in range(HC):
                    ct_ap = ct_q[hc // 4][:, (hc % 4) * 512:(hc % 4 + 1) * 512]
                    nc.tensor.matmul(kps[:], wk_sb[:, hc * D:(hc + 1) * D],
                                     ct_ap,
                                     start=(hc == 0), stop=(hc == HC - 1))
                    nc.tensor.matmul(vps[:], wv_sb[:, hc * D:(hc + 1) * D],
                                     ct_ap,
                                     start=(hc == 0), stop=(hc == HC - 1))
                vT_sb = small.tile([P, 512], BF16, name="vT", tag="vT")
                nc.vector.tensor_copy(vT_sb[:], vps[:])
                ksq = small.tile([P, 512], BF16, name="ksq", tag="sq")
                nc.scalar.square(ksq[:], kps[:])
                ksum = t_wop("ksum")
                nc.tensor.matmul(ksum[:], ones_b[:], ksq[:],
                                 start=True, stop=True)
                krs = small.tile([P, 512], F32, name="krs", tag="rs")
                nc.scalar.activation(krs[:], ksum[:], AF.Sqrt,
                                     bias=eps_sb[:], scale=1.0 / D)
                krr = small.tile([P, 512], F32, name="krr", tag="rr")
                nc.vector.reciprocal_approx_fast(out=krr[:], in_=krs[:])
                nc.vector.scalar_tensor_tensor(
                    out=kT_sb[:, kcol * 512:(kcol + 1) * 512], in0=kps[:],
                    scalar=nkw_sb[:], in1=krr[:], op0=OP.mult, op1=OP.mult)

                def mk(j):
                    def transpose_one():
                        kt = kcol * 4 + j
                        tp = psum.tile([P, P], BF16, name="tp", tag="tp",
                                       bufs=1)
                        nc.tensor.transpose(tp[:],
                                            vT_sb[:, j * P:(j + 1) * P],
                                            ident[:])
                        nc.vector.tensor_copy(v_sb[:, kt * D:(kt + 1) * D],
                                              tp[:])
                    return transpose_one
                return [mk(j) for j in range(4)]

            # =========== interleaved A/B with need-ordered DMA ===========
            # All DMAs on the Sync hwdge queue, quarter-block (512KB)
            # granularity, issued in need-time order.
            def ct_dma(kcol):
                qs = []
                for q in range(4):
                    t = stream.tile([P, 4 * 512], BF16, name="ctq", tag="ct",
                                    bufs=6)
                    nc.sync.dma_start(
                        t[:], cT[kcol][:, q * 2048:(q + 1) * 2048])
                    qs.append(t)
                return qs

            def xt_dma(pb):
                qs = []
                for q in range(4):
                    t = stream.tile([P, 4 * 512], BF16, name="xtq", tag="xt",
                                    bufs=6)
                    nc.sync.dma_start(
                        t[:], xT[pb][:, q * 2048:(q + 1) * 2048])
                    qs.append(t)
                return qs

            def xt_ap_of(qs):
                return lambda hc: qs[hc // 4][:, (hc % 4) * PB:
                                              (hc % 4 + 1) * PB]

            xt0 = []
            for q in range(4):
                nc.sync.dma_start(wq_qs[q][:], wq[q])
                t = stream.tile([P, 4 * 512], BF16, name="xtq", tag="xt",
                                bufs=6)
                nc.sync.dma_start(t[:], xT[0][:, q * 2048:(q + 1) * 2048])
                xt0.append(t)
            nc.sync.dma_start(wk_sb[:], wk[:])
            nc.sync.dma_start(wv_sb[:], wv[:])

            ct0 = ct_dma(0)
            emit_A(0, xt_ap_of(xt0))
            xt1 = xt_dma(1)
            ct1 = ct_dma(1)
            tr0 = emit_B(0, ct0)
            emit_A(1, xt_ap_of(xt1), extra_pe=tr0)
            xt2 = xt_dma(2)
            ct2 = ct_dma(2)
            tr1 = emit_B(1, ct1)
            emit_A(2, xt_ap_of(xt2), extra_pe=tr1)
            ct3 = ct_dma(3)
            tr2 = emit_B(2, ct2)
            xt3 = xt_dma(3)
            nc.sync.dma_start(wo_sb[:], wo[:])
            tr3 = emit_B(3, ct3)
            emit_A(3, xt_ap_of(xt3), extra_pe=tr2 + tr3)

            # =========== Phase C: attention + wo (pipelined) ===========
            prev = None  # (q0, attn_map) of previous ab awaiting wo

            def wo_filler_gen(q0p, attn_map, tags=(t_wop,)):
                """Yields once per emitted wo matmul; every 4th closes a
                (qs, ht) chunk with copy + DMA out."""
                ci = 0
                for qs in range(4):
                    for ht in range(4):
                        wop = tags[ci % len(tags)]("wop")
                        ci += 1
                        for h in range(NH):
                            nc.tensor.matmul(
                                wop[:],
                                attn_map[h][:, qs * P:(qs + 1) * P],
                                wo_sb[:, h * HID + ht * 512:
                                      h * HID + (ht + 1) * 512],
                                start=(h == 0), stop=(h == NH - 1))
                            yield
                        ot = outp.tile([P, 512], F32, name="ot", tag="ot")
                        nc.vector.tensor_copy(ot[:], wop[:])
                        nc.sync.dma_start(
                            out[q0p + qs * P: q0p + (qs + 1) * P,
                                ht * 512:(ht + 1) * 512], ot[:])

            for ab in range(NAB):
                q0 = ab * AB
                filler = (wo_filler_gen(*prev) if prev is not None else None)
                attn_map = {}
                for h in range(NH):
                    attps = t_catt(f"attps{h}")
                    sumps = t_csum(f"sumps{h}")
                    for kt in range(KC):
                        st = t_st("st")
                        nc.tensor.matmul(st[:],
                                         kT_sb[:, kt * P:(kt + 1) * P],
                                         xqT_list[h][:, q0:q0 + AB],
                                         start=True, stop=True)
                        e = esbp.tile([P, AB], BF16, name="e", tag="e")
                        nc.scalar.activation(e[:], st[:], AF.Exp)
                        nc.tensor.matmul(sumps[:], ones_b[:], e[:],
                                         start=(kt == 0), stop=(kt == KC - 1))
                        nc.tensor.matmul(attps[:],
                                         v_sb[:, kt * D:(kt + 1) * D],
                                         e[:],
                                         start=(kt == 0), stop=(kt == KC - 1))
                        if filler is not None:
                            next(filler, None)
                    rr = small.tile([P, AB], F32, name="arr", tag="arr")
                    nc.vector.reciprocal_approx_fast(out=rr[:], in_=sumps[:])
                    attn = small.tile([P, AB], BF16, name="attn",
                                      tag=f"attn{h}", bufs=2)
                    nc.vector.tensor_tensor(out=attn[:], in0=attps[:],
                                            in1=rr[:], op=OP.mult)
                    attn_map[h] = attn
                if filler is not None:
                    for _ in filler:  # drain any remainder
                        pass
                prev = (q0, attn_map)

            # final ab's wo: rotate across three tags (no next block to
            # interleave into; avoids per-chunk copy-wait stalls)
            for _ in wo_filler_gen(prev[0], prev[1],
                                   tags=(t_wop, t_st, t_csum)):
                pass

    nc.compile()
    return nc


def _get_compiled():
    global _compiled
    if _compiled is None:
        _compiled = _build()
    return _compiled


def _to_sbuf_images(aT):
    """[HID, S] f32 -> [NPB, 128, HC*512] bf16 (per-block SBUF images)."""
    t = aT.reshape(HC, P, NPB, PB).transpose(2, 1, 0, 3).reshape(
        NPB, P, HC * PB)
    return np.ascontiguousarray(t.astype(ml_dtypes.bfloat16))


def _weight_image(w, ncols):
    """[HC*P rows, ncols] -> SBUF image [128, HC*ncols] bf16."""
    nchunk = w.shape[0] // P
    img = w.reshape(nchunk, P, ncols).transpose(1, 0, 2).reshape(P, nchunk * ncols)
    return np.ascontiguousarray(img.astype(ml_dtypes.bfloat16))


def _shard_inputs(x, c, wq, wkv, wo, norm_q_w, norm_k_w):
    x = np.asarray(x, np.float32)
    c = np.asarray(c, np.float32)
    wq = np.asarray(wq, np.float32)
    wkv = np.asarray(wkv, np.float32)
    wo = np.asarray(wo, np.float32)
    nqw = (np.asarray(norm_q_w, np.float32) * np.float32(SCALE)).reshape(P, 1)
    nkw = np.asarray(norm_k_w, np.float32).reshape(P, 1).copy()

    xTs = [_to_sbuf_images(x[b].T) for b in range(B)]
    cTs = [_to_sbuf_images(c[b].T) for b in range(B)]
    in_maps = []
    for core in range(8):
        b, g = core // 4, core % 4
        blk = wkv[:, g * 256:(g + 1) * 256]
        wq_sh = wq[:, g * 512:(g + 1) * 512]
        in_maps.append({
            "xT": xTs[b],
            "cT": cTs[b],
            "wq": np.ascontiguousarray(
                wq_sh.reshape(4, 4, P, NH * D).transpose(0, 2, 1, 3)
                .reshape(4, P, 4 * NH * D).astype(ml_dtypes.bfloat16)),
            "wk": _weight_image(np.ascontiguousarray(blk[:, 0::2]), D),
            "wv": _weight_image(np.ascontiguousarray(blk[:, 1::2]), D),
            "wo": _weight_image(wo[g * 512:(g + 1) * 512, :], HID),
            "nqw": nqw,
            "nkw": nkw,
        })
    return in_maps


def run_sharded(inputs, trace=False, trace_cores=None):
    """Run the SPMD kernel; returns (full_output, BassKernelResults)."""
    nc = _get_compiled()
    in_maps = _shard_inputs(**inputs)
    res = run_bass_kernel_spmd(nc, in_maps, core_ids=list(range(8)),
                               trace=trace, trace_cores=trace_cores)
    parts = [r["out"] for r in res.results]
    full = np.empty((B, S, HID), np.float32)
    for b in range(B):
        full[b] = np.sum(np.stack([parts[4 * b + g] for g in range(4)], 0),
                         axis=0, dtype=np.float64).astype(np.float32)
    return full, res


def kernel(**inputs) -> np.ndarray:
    out, _ = run_sharded(inputs, trace=False)
    return out


# revision 12
# speedup vs baseline: 1.3303x; 1.0078x over previous
"""Trainium2 Bass kernel for nn_CrossAttention_51539607552970.

Sharding: 8 cores = 2 (batch) x 4 (GQA kv-head groups). Each core computes
4 query heads + its single kv head for one batch element, producing a
partial output (its head-group's contribution through wo); the host sums
the 4 partials per batch element (tensor-parallel unshard).

Host passes x/c transposed as per-block SBUF images ([128, 16*512] bf16,
one contiguous 2MB DMA per 512-token block) and weights as SBUF images.
All matmuls run in bf16 (full PE rate, f32 PSUM accumulate).

Schedule: Q-projection (A) and KV-projection (B) blocks interleave
(A0 B0 A1 B1 A2 B2 B3 A3) with DMAs issued in need-time order; compute
starts ~1us after the preamble.  Attention (C) runs single-head
pipelines; the output projection (wo) of the previous query block is
interleaved one matmul per kt step as PE filler so exp (scalar) latency
never stalls the PE; the final block's wo rotates across three PSUM
tags to avoid copy-wait stalls.  PSUM: catt 2 + csum 1 + st 3 + wop 1 +
tp 1 = 8 banks.
"""

import sys

sys.path.insert(0, "/opt/trn_rl_repo")

import ml_dtypes
import numpy as np

import concourse.bass as bass
import concourse.mybir as mybir
import concourse.tile as tile
from concourse import bacc
from concourse.bass_utils import run_bass_kernel_spmd
from concourse.masks import make_identity

F32 = mybir.dt.float32
BF16 = mybir.dt.bfloat16
AF = mybir.ActivationFunctionType
OP = mybir.AluOpType

# Problem constants (hardcoded per contract).
B, S, L = 2, 2048, 2048
H, KVH, D = 16, 4, 128
HID = H * D
EPS = 1e-6
SCALE = 1.0 / np.sqrt(D)

NH = 4           # query heads per core
P = 128          # partitions
HC = HID // P    # 16 hid chunks
KC = L // P      # 16 key chunks
PB = 512         # projection block width (tokens)
AB = 512         # attention block width (queries)
NPB = S // PB    # 4
NAB = S // AB    # 4

_compiled = None


def _build():
    nc = bacc.Bacc("TRN2", num_devices=8)

    # Per-block SBUF images: [128, HC*512] bf16, contiguous.
    xT = nc.dram_tensor("xT", [NPB, P, HC * PB], BF16, kind="ExternalInput")
    cT = nc.dram_tensor("cT", [4, P, HC * 512], BF16, kind="ExternalInput")
    wq = nc.dram_tensor("wq", [4, P, 4 * NH * D], BF16, kind="ExternalInput")
    wk = nc.dram_tensor("wk", [P, HC * D], BF16, kind="ExternalInput")
    wv = nc.dram_tensor("wv", [P, HC * D], BF16, kind="ExternalInput")
    wo = nc.dram_tensor("wo", [P, NH * HID], BF16, kind="ExternalInput")
    nqw = nc.dram_tensor("nqw", [P, 1], F32, kind="ExternalInput")
    nkw = nc.dram_tensor("nkw", [P, 1], F32, kind="ExternalInput")
    out = nc.dram_tensor("out", [S, HID], F32, kind="ExternalOutput")

    with nc.allow_low_precision(reason="bf16 matmul inputs"), \
         tile.TileContext(nc) as tc:
        with tc.tile_pool(name="consts", bufs=1) as consts, \
             tc.tile_pool(name="weights", bufs=1) as weights, \
             tc.tile_pool(name="stream", bufs=2) as stream, \
             tc.tile_pool(name="stream0", bufs=1) as stream0, \
             tc.tile_pool(name="kv", bufs=1) as kvpool, \
             tc.tile_pool(name="xqt", bufs=1) as xqtpool, \
             tc.tile_pool(name="small", bufs=2) as small, \
             tc.tile_pool(name="esbp", bufs=4) as esbp, \
             tc.tile_pool(name="outp", bufs=3) as outp, \
             tc.tile_pool(name="psum", bufs=1, space="PSUM") as psum:

            # ---- constants (no DMA deps) ----
            ones_f = consts.tile([P, P], F32)
            nc.vector.memset(ones_f[:], 1.0)
            ones_b = consts.tile([P, P], BF16)
            nc.scalar.copy(ones_b[:], ones_f[:])
            ident_f = consts.tile([P, P], F32)
            make_identity(nc, ident_f)
            ident = consts.tile([P, P], BF16)
            nc.scalar.copy(ident[:], ident_f[:])
            eps_sb = consts.tile([P, 1], F32)
            nc.vector.memset(eps_sb[:], EPS)

            nqw_sb = consts.tile([P, 1], F32)
            nkw_sb = consts.tile([P, 1], F32)
            nc.sync.dma_start(nqw_sb[:], nqw[:])
            nc.sync.dma_start(nkw_sb[:], nkw[:])

            # ---- weights ----
            wq_qs = [None] + [weights.tile([P, 4 * NH * D], BF16,
                                           name=f"wqq{j}")
                              for j in range(1, 4)]
            wk_sb = weights.tile([P, HC * D], BF16)
            wv_sb = weights.tile([P, HC * D], BF16)
            wo_sb = weights.tile([P, NH * HID], BF16)

            # ---- persistent activations ----
            kT_sb = kvpool.tile([P, L], BF16)              # [D, keys]
            v_sb = kvpool.tile([P, KC * D], BF16)          # kt-th blk [keys, D]
            xqT_list = [xqtpool.tile([P, S], BF16, name=f"xqT{h}")
                        for h in range(NH)]

            # PSUM tags (static banks): catt 2 + csum 1 + st 3 + wop 1 + tp 1
            def t_catt(nm):
                return psum.tile([P, 512], F32, name=nm, tag="catt", bufs=2)

            def t_csum(nm):
                return psum.tile([P, 512], F32, name=nm, tag="csum", bufs=1)

            def t_st(nm):
                return psum.tile([P, 512], F32, name=nm, tag="st", bufs=3)

            def t_wop(nm):
                return psum.tile([P, 512], F32, name=nm, tag="wop", bufs=1)

            def t_tp(nm):
                return psum.tile([P, 512], F32, name=nm, tag="tp", bufs=1)

            # ---------- phase-A block: Q projection for one pb ----------
            def emit_A(pb, xt_ap, extra_pe=()):
                # two 2-head passes so the norm chain of pass1 overlaps
                # pass2's matmuls (and pass2's norm overlaps the next block)
                extra_pe = list(extra_pe)
                slots = ([3, 7, 11, 15] if len(extra_pe) <= 4
                         else [1, 3, 5, 7, 9, 11, 13, 15])
                for hp in range(2):
                    hs = [2 * hp, 2 * hp + 1]
                    qpss = {h: (t_catt(f"qps{h}") if hp == 0
                                else t_csum(f"qps{h}") if h == 2
                                else t_tp(f"qps{h}")) for h in hs}
                    for hc in range(HC):
                        for h in hs:
                            wq_ap = wq_ap_g(hc, h)
                            nc.tensor.matmul(
                                qpss[h][:], wq_ap, xt_ap(hc),
                                start=(hc == 0), stop=(hc == HC - 1))
                        if extra_pe and hp == 0 and hc in slots:
                            extra_pe.pop(0)()
                    for h in hs:
                        qps = qpss[h]
                        qsq = small.tile([P, PB], BF16, name="qsq", tag="sq")
                        nc.scalar.square(qsq[:], qps[:])
                        qsum = t_wop("qsum")
                        nc.tensor.matmul(qsum[:], ones_b[:], qsq[:],
                                         start=True, stop=True)
                        qrs = small.tile([P, PB], F32, name="qrs", tag="rs")
                        nc.scalar.activation(qrs[:], qsum[:], AF.Sqrt,
                                             bias=eps_sb[:], scale=1.0 / D)
                        qrr = small.tile([P, PB], F32, name="qrr", tag="rr")
                        nc.vector.reciprocal_approx_fast(out=qrr[:], in_=qrs[:])
                        nc.vector.scalar_tensor_tensor(
                            out=xqT_list[h][:, pb * PB:(pb + 1) * PB],
                            in0=qps[:], scalar=nqw_sb[:], in1=qrr[:],
                            op0=OP.mult, op1=OP.mult)

            # ---------- phase-B block: K/V projection for one kcol ----------
            # returns closures emitting the 4 deferred V-transposes
            def emit_B(kcol, ct_q):
                kps = t_st("kps")
                vps = t_st("vps")
                for hc in range(HC):
                    ct_ap = ct_q[hc // 4][:, (hc % 4) * 512:(hc % 4 + 1) * 512]
                    nc.tensor.matmul(kps[:], wk_sb[:, hc * D:(hc + 1) * D],
                                     ct_ap,
                                     start=(hc == 0), stop=(hc == HC - 1))
                    nc.tensor.matmul(vps[:], wv_sb[:, hc * D:(hc + 1) * D],
                                     ct_ap,
                                     start=(hc == 0), stop=(hc == HC - 1))
                vT_sb = small.tile([P, 512], BF16, name="vT", tag="vT")
                nc.vector.tensor_copy(vT_sb[:], vps[:])
                ksq = small.tile([P, 512], BF16, name="ksq", tag="sq")
                nc.scalar.square(ksq[:], kps[:])
                ksum = t_wop("ksum")
                nc.tensor.matmul(ksum[:], ones_b[:], ksq[:],
                                 start=True, stop=True)
                krs = small.tile([P, 512], F32, name="krs", tag="rs")
                nc.scalar.activation(krs[:], ksum[:], AF.Sqrt,
                                     bias=eps_sb[:], scale=1.0 / D)
                krr = small.tile([P, 512], F32, name="krr", tag="rr")
                nc.vector.reciprocal_approx_fast(out=krr[:], in_=krs[:])
                nc.vector.scalar_tensor_tensor(
                    out=kT_sb[:, kcol * 512:(kcol + 1) * 512], in0=kps[:],
                    scalar=nkw_sb[:], in1=krr[:], op0=OP.mult, op1=OP.mult)

                def mk(j):
                    def transpose_one():
                        kt = kcol * 4 + j
                        tp = psum.tile([P, P], BF16, name="tp", tag="tp",
                                       bufs=1)
                        nc.tensor.transpose(tp[:],
                                            vT_sb[:, j * P:(j + 1) * P],
                                            ident[:])
                        nc.vector.tensor_copy(v_sb[:, kt * D:(kt + 1) * D],
                                              tp[:])
                    return transpose_one
                return [mk(j) for j in range(4)]

            # =========== interleaved A/B with need-ordered DMA ===========
            # All DMAs on the Sync hwdge queue, quarter-block (512KB)
            # granularity, issued in need-time order.
            def ct_dma(kcol):
                qs = []
                for q in range(4):
                    t = stream.tile([P, 4 * 512], BF16, name="ctq", tag="ct",
                                    bufs=6)
                    nc.sync.dma_start(
                        t[:], cT[kcol][:, q * 2048:(q + 1) * 2048])
                    qs.append(t)
                return qs

            def xt_dma(pb):
                qs = []
                for q in range(4):
                    t = stream.tile([P, 4 * 512], BF16, name="xtq", tag="xt",
                                    bufs=6)
                    nc.sync.dma_start(
                        t[:], xT[pb][:, q * 2048:(q + 1) * 2048])
                    qs.append(t)
                return qs

            def xt_ap_of(qs):
                return lambda hc: qs[hc // 4][:, (hc % 4) * PB:
                                              (hc % 4 + 1) * PB]

            # First quarter in halves so the very first matmul's
            # inputs land ~4us sooner.
            wq_h = [weights.tile([P, 2 * NH * D], BF16, name=f"wqh{j}")
                    for j in range(2)]
            xt0_h = [stream0.tile([P, 2 * PB], BF16, name=f"xt0h{j}")
                     for j in range(2)]
            for j in range(2):
                nc.sync.dma_start(wq_h[j][:],
                                  wq[0][:, j * 1024:(j + 1) * 1024])
                nc.sync.dma_start(xt0_h[j][:],
                                  xT[0][:, j * 1024:(j + 1) * 1024])
            xt0q = []
            for q in range(1, 4):
                nc.sync.dma_start(wq_qs[q][:], wq[q])
                t = stream.tile([P, 4 * 512], BF16, name="xtq", tag="xt",
                                bufs=6)
                nc.sync.dma_start(t[:], xT[0][:, q * 2048:(q + 1) * 2048])
                xt0q.append(t)
            nc.sync.dma_start(wk_sb[:], wk[:])
            nc.sync.dma_start(wv_sb[:], wv[:])

            def xt0_ap(hc):
                if hc < 4:
                    return xt0_h[hc // 2][:, (hc % 2) * PB:(hc % 2 + 1) * PB]
                return xt0q[hc // 4 - 1][:, (hc % 4) * PB:(hc % 4 + 1) * PB]

            def wq_ap_g(hc, h):
                if hc < 4:
                    return wq_h[hc // 2][:, (hc % 2) * 512 + h * D:
                                         (hc % 2) * 512 + (h + 1) * D]
                return wq_qs[hc // 4][:, (hc % 4) * 512 + h * D:
                                      (hc % 4) * 512 + (h + 1) * D]

            ct0 = ct_dma(0)
            emit_A(0, xt0_ap)
            xt1 = xt_dma(1)
            ct1 = ct_dma(1)
            tr0 = emit_B(0, ct0)
            emit_A(1, xt_ap_of(xt1), extra_pe=tr0)
            xt2 = xt_dma(2)
            ct2 = ct_dma(2)
            tr1 = emit_B(1, ct1)
            emit_A(2, xt_ap_of(xt2), extra_pe=tr1)
            xt3 = xt_dma(3)
            tr2 = emit_B(2, ct2)
            ct3 = ct_dma(3)
            emit_A(3, xt_ap_of(xt3), extra_pe=tr2)
            nc.sync.dma_start(wo_sb[:], wo[:])
            tr3 = emit_B(3, ct3)

            # =========== Phase C: attention + wo (pipelined) ===========
            prev = None  # (q0, attn_map) of previous ab awaiting wo

            def wo_filler_gen(q0p, attn_map, tags=(t_wop,), alt=False):
                """Yields once per emitted wo matmul; every 4th closes a
                (qs, ht) chunk with copy + DMA out.  With alt=True the
                copy/DMA alternate across engines (fast tail drain)."""
                ci = 0
                for qs in range(4):
                    for ht in range(4):
                        wop = tags[ci % len(tags)]("wop")
                        use_scalar = alt and (ci % 2 == 1)
                        ci += 1
                        for h in range(NH):
                            nc.tensor.matmul(
                                wop[:],
                                attn_map[h][:, qs * P:(qs + 1) * P],
                                wo_sb[:, h * HID + ht * 512:
                                      h * HID + (ht + 1) * 512],
                                start=(h == 0), stop=(h == NH - 1))
                            yield
                        ot = outp.tile([P, 512], F32, name="ot", tag="ot")
                        dst = out[q0p + qs * P: q0p + (qs + 1) * P,
                                  ht * 512:(ht + 1) * 512]
                        if use_scalar:
                            nc.scalar.copy(ot[:], wop[:])
                            nc.scalar.dma_start(dst, ot[:])
                        else:
                            nc.vector.tensor_copy(ot[:], wop[:])
                            nc.sync.dma_start(dst, ot[:])

            for ab in range(NAB):
                q0 = ab * AB
                filler = (wo_filler_gen(*prev) if prev is not None else None)
                attn_map = {}
                for h in range(NH):
                    attps = t_catt(f"attps{h}")
                    sumps = t_csum(f"sumps{h}")
                    for kt in range(KC):
                        st = t_st("st")
                        nc.tensor.matmul(st[:],
                                         kT_sb[:, kt * P:(kt + 1) * P],
                                         xqT_list[h][:, q0:q0 + AB],
                                         start=True, stop=True)
                        e = esbp.tile([P, AB], BF16, name="e", tag="e")
                        nc.scalar.activation(e[:], st[:], AF.Exp)
                        nc.tensor.matmul(sumps[:], ones_b[:], e[:],
                                         start=(kt == 0), stop=(kt == KC - 1))
                        nc.tensor.matmul(attps[:],
                                         v_sb[:, kt * D:(kt + 1) * D],
                                         e[:],
                                         start=(kt == 0), stop=(kt == KC - 1))
                        if ab == 0 and h == 0 and 4 <= kt < 8:
                            tr3[kt - 4]()
                        if filler is not None:
                            next(filler, None)
                    rr = small.tile([P, AB], F32, name="arr", tag="arr")
                    nc.vector.reciprocal_approx_fast(out=rr[:], in_=sumps[:])
                    attn = small.tile([P, AB], BF16, name="attn",
                                      tag=f"attn{h}", bufs=2)
                    nc.vector.tensor_tensor(out=attn[:], in0=attps[:],
                                            in1=rr[:], op=OP.mult)
                    attn_map[h] = attn
                if filler is not None:
                    for _ in filler:  # drain any remainder
                        pass
                prev = (q0, attn_map)

            # final ab's wo: rotate across three tags (no next block to
            # interleave into; avoids per-chunk copy-wait stalls)
            for _ in wo_filler_gen(prev[0], prev[1],
                                   tags=(t_wop, t_st, t_csum), alt=True):
                pass

    nc.compile()
    return nc


def _get_compiled():
    global _compiled
    if _compiled is None:
        _compiled = _build()
    return _compiled


def _to_sbuf_images(aT):
    """[HID, S] f32 -> [NPB, 128, HC*512] bf16 (per-block SBUF images)."""
    t = aT.reshape(HC, P, NPB, PB).transpose(2, 1, 0, 3).reshape(
        NPB, P, HC * PB)
    return np.ascontiguousarray(t.astype(ml_dtypes.bfloat16))


def _weight_image(w, ncols):
    """[HC*P rows, ncols] -> SBUF image [128, HC*ncols] bf16."""
    nchunk = w.shape[0] // P
    img = w.reshape(nchunk, P, ncols).transpose(1, 0, 2).reshape(P, nchunk * ncols)
    return np.ascontiguousarray(img.astype(ml_dtypes.bfloat16))


def _shard_inputs(x, c, wq, wkv, wo, norm_q_w, norm_k_w):
    x = np.asarray(x, np.float32)
    c = np.asarray(c, np.float32)
    wq = np.asarray(wq, np.float32)
    wkv = np.asarray(wkv, np.float32)
    wo = np.asarray(wo, np.float32)
    nqw = (np.asarray(norm_q_w, np.float32) * np.float32(SCALE)).reshape(P, 1)
    nkw = np.asarray(norm_k_w, np.float32).reshape(P, 1).copy()

    xTs = [_to_sbuf_images(x[b].T) for b in range(B)]
    cTs = [_to_sbuf_images(c[b].T) for b in range(B)]
    in_maps = []
    for core in range(8):
        b, g = core // 4, core % 4
        blk = wkv[:, g * 256:(g + 1) * 256]
        wq_sh = wq[:, g * 512:(g + 1) * 512]
        in_maps.append({
            "xT": xTs[b],
            "cT": cTs[b],
            "wq": np.ascontiguousarray(
                wq_sh.reshape(4, 4, P, NH * D).transpose(0, 2, 1, 3)
                .reshape(4, P, 4 * NH * D).astype(ml_dtypes.bfloat16)),
            "wk": _weight_image(np.ascontiguousarray(blk[:, 0::2]), D),
            "wv": _weight_image(np.ascontiguousarray(blk[:, 1::2]), D),
            "wo": _weight_image(wo[g * 512:(g + 1) * 512, :], HID),
            "nqw": nqw,
            "nkw": nkw,
        })
    return in_maps


def run_sharded(inputs, trace=False, trace_cores=None):
    """Run the SPMD kernel; returns (full_output, BassKernelResults)."""
    nc = _get_compiled()
    in_maps = _shard_inputs(**inputs)
    res = run_bass_kernel_spmd(nc, in_maps, core_ids=list(range(8)),
                               trace=trace, trace_cores=trace_cores)
    parts = [r["out"] for r in res.results]
    full = np.empty((B, S, HID), np.float32)
    for b in range(B):
        full[b] = np.sum(np.stack([parts[4 * b + g] for g in range(4)], 0),
                         axis=0, dtype=np.float64).astype(np.float32)
    return full, res


def kernel(**inputs) -> np.ndarray:
    out, _ = run_sharded(inputs, trace=False)
    return out
